# revision 2
# baseline (speedup 1.0000x reference)
"""Trainium2 Bass kernel for nn_EuclideanEmbedding (vq_codebook).

reference:
    distances = cdist(x, p)                      # (8192, 512)
    r1 = mean_j min_i distances[i, j]            # scalar
    r2 = mean_i min_j distances[i, j]            # scalar

Strategy (8 NeuronCores):
  - Shard x along batch: 1024 rows/core; replicate the (512, 64) codebook.
  - On-device, per core: squared distances via ONE augmented matmul per
    128-row tile:  D2 = [x, ||x||^2, 1] @ [-2 p, 1, ||p||^2]^T   (K = 66)
    ScalarE takes sqrt(D2) PSUM->SBUF, DMA streams the distance shard out.
    VectorE computes per-row min (r2 partial) and a running elementwise
    column min (r1 partial) directly on the PSUM D2 tiles.
  - Host combines the per-core partial minima (the cross-device min /
    mean all-reduce of the sharding hint) while unsharding.
"""

import numpy as np

import concourse.bacc as bacc
import concourse.tile as tile
from concourse import mybir
from concourse.bass_utils import run_bass_kernel_spmd

BATCH = 8192
NV = 512          # codebook vectors
D = 64            # latent dim
NCORES = 8
BPC = BATCH // NCORES   # 1024 batch rows per core
P = 128                 # partitions
NTILES = BPC // P       # 8 tiles of 128 batch rows per core
KAUG = D + 2            # augmented contraction dim

_CACHE = {}


def _make_nc():
    return bacc.Bacc(
        "TRN2",
        target_bir_lowering=False,
        debug=False,
        enable_asserts=False,
        num_devices=NCORES,
    )


def _declare_io(nc):
    f32 = mybir.dt.float32
    return {
        "xaugT": nc.dram_tensor("xaugT", [KAUG, BPC], f32, kind="ExternalInput").ap(),
        "paugT": nc.dram_tensor("paugT", [KAUG, NV], f32, kind="ExternalInput").ap(),
        "dist": nc.dram_tensor("dist", [BPC, NV], f32, kind="ExternalOutput").ap(),
        "colmin": nc.dram_tensor("colmin", [P, NV], f32, kind="ExternalOutput").ap(),
        "rowmin": nc.dram_tensor("rowmin", [P, NTILES], f32, kind="ExternalOutput").ap(),
    }


def _emit_body(nc, io, singles, dists, psums, xaugT_sb, paugT_sb):
    """One full pass: 8 x [128, 512] distance tiles + partial mins."""
    f32 = mybir.dt.float32
    colmin_sb = singles.tile([P, NV], f32)
    rowmin_sb = singles.tile([P, NTILES], f32)
    for t in range(NTILES):
        psum_t = psums.tile([P, NV], f32)
        nc.tensor.matmul(
            psum_t[:],
            xaugT_sb[:, t * P:(t + 1) * P],
            paugT_sb[:],
            start=True,
            stop=True,
        )
        dist_sb = dists.tile([P, NV], f32)
        nc.scalar.sqrt(dist_sb[:], psum_t[:])
        nc.sync.dma_start(out=io["dist"][t * P:(t + 1) * P, :], in_=dist_sb[:])
        # r2 partial: min over the 512 codebook entries for each row
        nc.vector.tensor_reduce(
            rowmin_sb[:, t:t + 1],
            psum_t[:],
            axis=mybir.AxisListType.X,
            op=mybir.AluOpType.min,
        )
        # r1 partial: running elementwise min across batch tiles
        if t == 0:
            nc.vector.tensor_copy(colmin_sb[:], psum_t[:])
        else:
            nc.vector.tensor_tensor(
                colmin_sb[:], psum_t[:], colmin_sb[:], op=mybir.AluOpType.min,
            )
    nc.sync.dma_start(out=io["colmin"][:], in_=colmin_sb[:])
    nc.sync.dma_start(out=io["rowmin"][:], in_=rowmin_sb[:])


def _build_program(outer_loop=None, inner_unroll=1):
    """outer_loop=None -> single-pass production program.
    outer_loop=K -> For_i hardware loop with inner_unroll python-unrolled
    passes per iteration (timing amplification)."""
    f32 = mybir.dt.float32
    nc = _make_nc()
    io = _declare_io(nc)

    with tile.TileContext(nc) as tc:
        with (
            tc.tile_pool(name="singles", bufs=2) as singles,
            tc.tile_pool(name="dists", bufs=3) as dists,
            tc.tile_pool(name="psums", bufs=4, space="PSUM") as psums,
        ):
            xaugT_sb = singles.tile([KAUG, BPC], f32)
            paugT_sb = singles.tile([KAUG, NV], f32)
            nc.sync.dma_start(out=xaugT_sb[:], in_=io["xaugT"][:])
            nc.sync.dma_start(out=paugT_sb[:], in_=io["paugT"][:])

            if outer_loop is None:
                _emit_body(nc, io, singles, dists, psums, xaugT_sb, paugT_sb)
            else:
                with tc.For_i(0, outer_loop, 1):
                    for _ in range(inner_unroll):
                        _emit_body(nc, io, singles, dists, psums,
                                   xaugT_sb, paugT_sb)

    nc.compile()
    return nc


def _get_program():
    if "nc" not in _CACHE:
        _CACHE["nc"] = _build_program()
    return _CACHE["nc"]


def _prep_inputs(x, p):
    x = np.ascontiguousarray(np.asarray(x, dtype=np.float32))
    p = np.ascontiguousarray(np.asarray(p, dtype=np.float32))
    xsq = np.einsum("id,id->i", x.astype(np.float64), x.astype(np.float64))
    psq = np.einsum("jd,jd->j", p.astype(np.float64), p.astype(np.float64))
    xaugT = np.empty((KAUG, BATCH), np.float32)
    xaugT[:D] = x.T
    xaugT[D] = xsq.astype(np.float32)
    xaugT[D + 1] = 1.0
    paugT = np.empty((KAUG, NV), np.float32)
    paugT[:D] = -2.0 * p.T
    paugT[D] = 1.0
    paugT[D + 1] = psq.astype(np.float32)
    in_maps = []
    for c in range(NCORES):
        in_maps.append({
            "xaugT": np.ascontiguousarray(xaugT[:, c * BPC:(c + 1) * BPC]),
            "paugT": paugT,
        })
    return in_maps


def _run(x, p, trace=False, nc=None, **kwargs):
    if nc is None:
        nc = _get_program()
    in_maps = _prep_inputs(x, p)
    return run_bass_kernel_spmd(
        nc, in_maps, core_ids=list(range(NCORES)), trace=trace, **kwargs
    )


def _assemble(results):
    dist_full = np.empty((BATCH, NV), np.float32)
    colmins = np.empty((NCORES, P, NV), np.float32)
    rowmins = np.empty((NCORES, P, NTILES), np.float32)
    for c in range(NCORES):
        r = results[c]
        dist_full[c * BPC:(c + 1) * BPC] = r["dist"]
        colmins[c] = r["colmin"]
        rowmins[c] = r["rowmin"]
    # cross-core all-reduce-min over batch, then mean over codebook
    colmin_d2 = colmins.min(axis=(0, 1)).astype(np.float64)      # (512,)
    r1 = np.float32(np.mean(np.sqrt(colmin_d2)))
    # per-sample min is already complete locally; batch-mean all-reduce
    r2 = np.float32(np.mean(np.sqrt(rowmins.astype(np.float64))))
    return dist_full, r1, r2


def kernel(x, trainable_p):
    res = _run(x, trainable_p)
    return _assemble(res.results)
